# revision 1
# baseline (speedup 1.0000x reference)
"""BiDAF attention-flow layer on 8 Trainium2 NeuronCores.

Data-parallel over batch: each core processes B/8 = 8 batches.

Math (per batch b):
  s[t,j] = h[t]·w_h + u[j]·w_u + (h[t]*w_hu)·u[j] + const
  a      = softmax_j(s)            -> only needs  sj = shu + su  (row consts cancel)
  c2q    = a @ u
  bt     = softmax_t(max_j s)      -> needs  m + sh  where m = max_j(sj)
  q2c    = bt @ h
  g      = [h | c2q | h*c2q | h*q2c]

The rank-1 bias terms b_h/b_u/b_hu shift every s[t,j] equally and cancel in
both softmaxes, so they are accepted but unused.

Schedule (the kernel is DMA-bound at ~72.3us of transfer per core on the
cost model's shared 360 B/ns ring; everything else hides behind it):
  * all 8 batches' h tiles live in SBUF; loads for 4 batches go out first,
    later loads weave between stores so the DMA ring never starves,
  * per batch, issue order software-pipelines prep of batch b+1 (u-prep,
    hT transpose groups) between the dependent pair-stages of batch b,
    because every engine executes its instructions strictly in issue order,
  * g cols 0:200 stream straight from htile (no compute, issued on the ACT
    HWDGE queue so they never queue behind a compute-gated store on SP),
  * q2c accumulates TRANSPOSED (yT[d,1] += h.T e per chunk; out free-size 1
    makes these matmuls ~free vs 333ns for row-accumulation), with sum(e)
    riding htile's embedded ones column; softmax denominators come from
    ones-matmuls on the PE instead of DVE reductions.
PSUM (8 banks): s2 ring 2, hT/p transposes ring 2, c2q+rowsum ring 2,
yT/q2cb accumulators ring 2.  HARD-LEARNED RULE: two multi-instruction
PSUM accumulation groups must never interleave in one bank — the second
group's writes corrupt the first on hardware, and only on hardware.
"""
import sys

if '/opt/trn_rl_repo' not in sys.path:
    sys.path.insert(0, '/opt/trn_rl_repo')

import numpy as np

B, T, J, D = 64, 800, 50, 200
NCORES = 8
BC = B // NCORES            # batches per core
P = 128
TCHUNKS = [(c * P, min(P, T - c * P)) for c in range((T + P - 1) // P)]
KCHUNKS = [(0, 100), (100, 100)]
NPAD = 256
DS = 201  # htile chunk stride: 200 h cols + a ones column

_cache = {}
F32R = False  # f32r c2q is ~8us faster but 50x less accurate; keep exact


def _split_multi_waits(nc, max_waits=1):
    """This walrus build accepts at most one sync-wait per instruction.
    For any instruction carrying more, move the extra waits onto pure-wait
    EventSemaphore carriers inserted just before it on the same engine —
    the sequencer dispatches in order, so the blocking behavior is
    identical."""
    from concourse import mybir
    import bass_rust
    n = 0
    for f in nc.m.functions:
        for blk in f.blocks:
            insts = blk.instructions
            i = 0
            while i < len(insts):
                inst = insts[i]
                si = inst.sync_info
                if si is not None and len(si.on_wait) > max_waits:
                    waits = list(si.on_wait)
                    keep = waits[-max_waits:]
                    new = []
                    for w in waits[:-max_waits]:
                        d = mybir.InstEventSemaphore(
                            name=f"{inst.name}-sw{n}", ins=[], outs=[])
                        n += 1
                        d.engine = inst.engine
                        d.sync_info = bass_rust.SyncInfo(on_wait=[w], on_update=[])
                        new.append(d)
                    inst.sync_info = bass_rust.SyncInfo(
                        on_wait=keep, on_update=list(si.on_update))
                    for j, d in enumerate(new):
                        insts.insert(i + j, d)
                    i += len(new)
                i += 1
    return n


def _build(reps=1):
    import concourse.bass as bass
    import concourse.tile as tile
    from concourse import mybir, masks
    from contextlib import ExitStack

    f32 = mybir.dt.float32
    f32r = mybir.dt.float32r
    AF = mybir.ActivationFunctionType
    AX = mybir.AxisListType

    nc = bass.Bass()
    h_in = nc.declare_dram_parameter("h", [BC, T, D], f32, isOutput=False)
    u_in = nc.declare_dram_parameter("u", [BC, J, D], f32, isOutput=False)
    wh_in = nc.declare_dram_parameter("w_h", [D], f32, isOutput=False)
    wu_in = nc.declare_dram_parameter("w_u", [D], f32, isOutput=False)
    whu_in = nc.declare_dram_parameter("w_hu", [D], f32, isOutput=False)
    g_out = nc.declare_dram_parameter("g", [BC, T, 4 * D], f32, isOutput=True)

    with tile.TileContext(nc) as tc, ExitStack() as ctx:
        singles = ctx.enter_context(tc.tile_pool(name="singles", bufs=1))
        hpool = ctx.enter_context(tc.tile_pool(name="hpool", bufs=BC))
        hTpool = ctx.enter_context(tc.tile_pool(name="hTpool", bufs=3))
        upool = ctx.enter_context(tc.tile_pool(name="upool", bufs=4))
        uldpool = ctx.enter_context(tc.tile_pool(name="uldpool", bufs=BC))
        bsmall = ctx.enter_context(tc.tile_pool(name="bsmall", bufs=4))
        csmall = ctx.enter_context(tc.tile_pool(name="csmall", bufs=6))
        gpool = ctx.enter_context(tc.tile_pool(name="gpool", bufs=8))
        hqpool = ctx.enter_context(tc.tile_pool(name="hqpool", bufs=3))
        ps_s = ctx.enter_context(
            tc.tile_pool(name="ps_s", bufs=2, space=bass.MemorySpace.PSUM))
        ps_tp = ctx.enter_context(
            tc.tile_pool(name="ps_tp", bufs=2, space=bass.MemorySpace.PSUM))
        ps_c2q = ctx.enter_context(
            tc.tile_pool(name="ps_c2q", bufs=2, space=bass.MemorySpace.PSUM))
        ps_acc = ctx.enter_context(
            tc.tile_pool(name="ps_acc", bufs=2, space=bass.MemorySpace.PSUM))

        # ---- once-per-core constants ----
        identity = singles.tile([P, P], f32)
        masks.make_identity(nc, identity[:])
        ones_row = singles.tile([1, P], f32)
        nc.vector.memset(ones_row, 1.0)
        ones_col = singles.tile([P, 1], f32)
        nc.vector.memset(ones_col, 1.0)

        wh0 = singles.tile([96, 1], f32, tag="wh0", name="wh0")
        wh1e = singles.tile([105, 1], f32, tag="wh1e", name="wh1e")
        nc.vector.memset(wh1e, 0.0)
        whu_b = singles.tile([J, D], f32, tag="whu_b", name="whu_b")
        wu_b = singles.tile([J, D], f32, tag="wu_b", name="wu_b")

        def load_weights_a():
            # whu/wu first — they gate the u-prep chain
            for t_, wsrc in ((whu_b, whu_in), (wu_b, wu_in)):
                s_ap = wsrc[:]
                nc.scalar.dma_start(out=t_[:, :], in_=bass.AP(
                    tensor=s_ap.tensor, offset=s_ap.offset,
                    ap=[[0, J], s_ap.ap[0][:]]))

        def load_weights_b():
            nc.scalar.dma_start(out=wh0[:, :],
                              in_=wh_in[0:96].rearrange("(p one) -> p one",
                                                        one=1))
            nc.scalar.dma_start(out=wh1e[0:104, :],
                              in_=wh_in[96:D].rearrange("(p one) -> p one",
                                                        one=1))

        def warm_pe():
            # keep the PE busy through its p-state ramp window so batch 0's
            # transposes run at full clock when h arrives
            for w in range(8):
                scr = ps_c2q.tile([P, DS + 1], f32, tag="c2q", name="scr")
                nc.tensor.matmul(scr[:J, 0:P], lhsT=identity[:P, 0:J],
                                 rhs=identity[:P, :P], is_transpose=True,
                                 skip_group_check=True)

        loaded = {}

        upair = {}

        def load_body(b):
            # u loads fused per batch-pair: one [50, 2, 200] DMA per two
            # batches halves the startup HWDGE/dispatch count
            if b % 2 == 0:
                u2 = uldpool.tile([J, 2 * D], f32, tag="u", name="u2")
                nc.sync.dma_start(
                    out=u2[:, :].rearrange("j (b d) -> j b d", d=D),
                    in_=u_in[b:b + 2, :, :].rearrange("b j d -> j b d"))
                upair[b] = u2
            u_sb = upair[b - b % 2][:, (b % 2) * D:(b % 2 + 1) * D]
            htile = hpool.tile([P, 7 * DS], f32, tag="h", name="htile")
            nc.gpsimd.memset(
                htile[:, :].rearrange("p (n d) -> p n d", d=DS)[:, :, D:DS], 1.0)
            nmain = 6
            nc.sync.dma_start(
                out=htile[:, 0:nmain * DS].rearrange(
                    "p (n d) -> p n d", d=DS)[:, :, 0:D],
                in_=h_in[b, 0:nmain * P, :].rearrange("(n p) d -> p n d", p=P),
            )
            nc.sync.dma_start(
                out=htile[0:T - nmain * P, nmain * DS:nmain * DS + D],
                in_=h_in[b, nmain * P:T, :],
            )
            loaded[b] = (u_sb, htile)

        def stage_uprep(b, S):
            u_sb, htile = loaded.pop(b)
            S['u_sb'], S['htile'] = u_sb, htile

            # ---- u-side prep: u_w = u*w_hu with su appended as col 200;
            # transposing u_w gives the s-matmul rhs (su lands in K-row 104,
            # paired with the ones column embedded in htile) ----
            u_w = upool.tile([J, DS], f32, tag="u_w", name="u_w")
            nc.vector.tensor_mul(out=u_w[:, 0:D], in0=u_sb[:, 0:D],
                                 in1=whu_b[:, :])
            su_t = upool.tile([J, D], f32, tag="su_t", name="su_t")
            nc.vector.tensor_mul(out=su_t[:, :], in0=u_sb[:, 0:D], in1=wu_b[:, :])
            # su written straight into u_w col 200 by the reduce (no extra hop)
            nc.vector.reduce_sum(out=u_w[:, D:DS], in_=su_t[:, :], axis=AX.X)

            rhs_ext = []
            for kc, (d0, kn, whcol) in enumerate(((0, 96, wh0), (96, 105, wh1e))):
                tp = ps_tp.tile([105, P], f32, tag="tp")
                nc.tensor.transpose(tp[:kn, :J], u_w[:J, d0:d0 + kn],
                                    identity[:J, :J])
                re_ = upool.tile([105, J + 1], f32, tag=f"rhs{kc}", name=f"rhs{kc}")
                nc.scalar.copy(out=re_[:kn, 0:J], in_=tp[:kn, :J])
                nc.vector.tensor_copy(out=re_[:kn, J:J + 1], in_=whcol[:kn, :])
                rhs_ext.append(re_)
            S['rhs_ext'] = rhs_ext
            S['hT'] = hTpool.tile([105, 2 * T], f32, tag="hT", name="hT")
            S['e_all'] = bsmall.tile([P, 7], f32, tag="e_all", name="e_all")
            S['msh_all'] = bsmall.tile([P, 7], f32, tag="msh_all",
                                       name="msh_all")
            # yT accumulator: col0 = sum_t e_t h_t[0:128]; col1 rows 0:72 =
            # d 128:200 plus row 72 = sum_t e_t (via htile's embedded ones col)
            S['yT_ps'] = ps_acc.tile([P, 1], f32, tag="accA", bufs=2,
                                     name="yT_ps")
            S['yT2_ps'] = ps_acc.tile([P, 1], f32, tag="accA", bufs=2,
                                      name="yT2_ps")
            S['yT_sb'] = bsmall.tile([P, 2], f32, tag="yT", name="yT_sb")

        def stage_hT(b, S, groups):
            # ---- h transpose: hT [105, 2*800]; kc1 row 104 = ones (su).
            # Two chunks' transposes (4 matmuls) share one PSUM tile and
            # drain with a single permuting copy. ----
            htile, hT = S['htile'], S['hT']
            for cpair in groups:
                t0 = cpair[0] * P
                ck = len(cpair)
                rows = TCHUNKS[cpair[-1]][1]
                tp = ps_tp.tile([105, 4 * P], f32, tag="tp")
                for ci, c in enumerate(cpair):
                    for kc, d0 in enumerate((0, 96)):
                        nc.tensor.matmul(
                            tp[:105, (2 * ci + kc) * P:(2 * ci + kc) * P + rows],
                            lhsT=htile[:rows, c * DS + d0:c * DS + d0 + 105],
                            rhs=identity[:rows, :rows], is_transpose=True,
                            skip_group_check=True)
                # src [p, (c k x)] -> dest hT [p, (k t)] with t = c*128 + x
                if ck == 2:
                    nc.scalar.copy(
                        out=hT[:105, :].rearrange("p (k t) -> p k t", k=2)
                        [:, :, t0:t0 + 2 * P].rearrange(
                            "p k (c x) -> p k c x", x=P),
                        in_=tp[:105, :].rearrange(
                            "p (c k x) -> p k c x", k=2, x=P))
                else:
                    nc.scalar.copy(
                        out=hT[:105, :].rearrange("p (k t) -> p k t", k=2)
                        [:, :, t0:t0 + rows],
                        in_=tp[:105, 0:2 * P].rearrange(
                            "p (k x) -> p k x", k=2)[:, :, :rows])

        def stage_pair(b, S, pair):
            # ---- chunks processed in groups (two pairs + one triple) so the
            # softmax reductions/exp run as one [128, k, 50] op each; the
            # 32-row tail chunk rides the last group (its garbage rows are
            # computed but never read or stored) ----
            u_sb, htile = S['u_sb'], S['htile']
            hT, rhs_ext = S['hT'], S['rhs_ext']
            e_all, yT_ps = S['e_all'], S['yT_ps']
            msh_all, yT_sb = S['msh_all'], S['yT_sb']
            yT2_ps = S['yT2_ps']
            c0 = pair[0]
            k = len(pair)
            t00 = c0 * P
            lrows = TCHUNKS[pair[-1]][1]   # rows of the last chunk
            nfull = k if lrows == P else k - 1   # chunks with all 128 rows
            s2 = ps_s.tile([P, 3 * (J + 1)], f32, tag="s", name="s2")
            for i, c in enumerate(pair):
                t0, rows = TCHUNKS[c]
                so = i * (J + 1)
                nc.tensor.matmul(s2[:rows, so:so + J + 1],
                                 lhsT=hT[0:96, t0:t0 + rows],
                                 rhs=rhs_ext[0][:96, :], start=True, stop=False)
                nc.tensor.matmul(s2[:rows, so:so + J + 1],
                                 lhsT=hT[0:105, T + t0:T + t0 + rows],
                                 rhs=rhs_ext[1][:105, :], start=False, stop=True)

            s2v = s2[:P, :].rearrange("p (k j) -> p k j", j=J + 1)[:, 0:k, :]
            m2 = csmall.tile([P, 3], f32, tag="m")
            nc.vector.reduce_max(out=m2[:P, 0:k], in_=s2v[:, :, 0:J], axis=AX.X)
            nc.vector.tensor_add(
                out=msh_all[:P, c0:c0 + k].rearrange(
                    "p (k one) -> p k one", one=1),
                in0=m2[:P, 0:k].rearrange("p (k one) -> p k one", one=1),
                in1=s2v[:, :, J:J + 1])
            p2 = csmall.tile([P, 3 * J], f32, tag="p", name="p2")
            nc.scalar.activation(
                out=p2[:P, 0:k * J].rearrange("p (k j) -> p k j", j=J),
                in_=s2v[:, :, 0:J], func=AF.Exp)
            tp2 = ps_tp.tile([100, 3 * P], f32, tag="tp")
            for i, c in enumerate(pair):
                rows = TCHUNKS[c][1]
                nc.tensor.matmul(tp2[:J, i * P:i * P + rows],
                                 lhsT=p2[:rows, i * J:(i + 1) * J],
                                 rhs=identity[:rows, :rows], is_transpose=True,
                                 skip_group_check=True)
            pT2 = csmall.tile([J, 3 * P], f32, tag="pT", name="pT2")
            nc.scalar.copy(out=pT2[:J, 0:(k - 1) * P + lrows],
                           in_=tp2[:J, 0:(k - 1) * P + lrows])

            # g cols 0:200 (= h) stream straight from htile — always-ready
            # DMA work (batch 0's went out early via stage_hstores)
            if b > 0:
                nc.scalar.dma_start(
                    out=g_out[b, t00:t00 + nfull * P, 0:D].rearrange(
                        "(k p) x -> p k x", p=P),
                    in_=htile[:, c0 * DS:(c0 + nfull) * DS].rearrange(
                        "p (k d) -> p k d", d=DS)[:, :, 0:D])
                if nfull < k:
                    nc.scalar.dma_start(
                        out=g_out[b, (c0 + nfull) * P:T, 0:D],
                        in_=htile[:lrows,
                                  (c0 + nfull) * DS:(c0 + nfull) * DS + D])
            gt = gpool.tile([P, 6 * D], f32, tag="g", name="gt")
            gv = gt[:P, 0:k * 2 * D].rearrange("p (k x) -> p k x", x=2 * D)
            hv = htile[:P, c0 * DS:(c0 + k) * DS].rearrange(
                "p (k d) -> p k d", d=DS)[:, :, 0:D]
            rcp2 = csmall.tile([P, 3], f32, tag="rcp")
            for i, c in enumerate(pair):
                t0, rows = TCHUNKS[c]
                cps = ps_c2q.tile([P, DS + 1], f32, tag="c2q")
                nc.tensor.matmul(cps[:rows, 0:D],
                                 lhsT=pT2[:J, i * P:i * P + rows],
                                 rhs=u_sb[:J, 0:D], start=True, stop=True,
                                 skip_group_check=True)
                # softmax denominator via ones-matmul (PE, free-size 1)
                nc.tensor.matmul(cps[:rows, D:D + 1],
                                 lhsT=pT2[:J, i * P:i * P + rows],
                                 rhs=ones_col[:J, :1], start=True, stop=True,
                                 skip_group_check=True)
                nc.vector.reciprocal(out=rcp2[:rows, i:i + 1],
                                     in_=cps[:rows, D:D + 1])
                nc.vector.tensor_scalar_mul(
                    out=gt[:rows, i * 2 * D:i * 2 * D + D],
                    in0=cps[:rows, 0:D], scalar1=rcp2[:rows, i:i + 1])
            # e = exp(m + sh) consolidated into two ACT ops per batch;
            # the yT accumulation matmuls are ~free (out free-size 1)
            if c0 in (2, 4):
                ec = (0, 4) if c0 == 2 else (4, 7)
                nc.scalar.activation(out=e_all[:, ec[0]:ec[1]],
                                     in_=msh_all[:, ec[0]:ec[1]],
                                     func=AF.Exp)
                # the two yT groups accumulate in parallel — in two
                # DIFFERENT banks (ring slots): open multi-instruction
                # groups must never share a PSUM bank
                for c in range(ec[0], ec[1]):
                    rows = TCHUNKS[c][1]
                    nc.tensor.matmul(yT_ps[0:P, 0:1],
                                     lhsT=htile[:rows, c * DS:c * DS + P],
                                     rhs=e_all[:rows, c:c + 1],
                                     start=(c == 0), stop=(c == 6),
                                     skip_group_check=True)
                    nc.tensor.matmul(yT2_ps[0:DS - P, 0:1],
                                     lhsT=htile[:rows,
                                                c * DS + P:c * DS + DS],
                                     rhs=e_all[:rows, c:c + 1],
                                     start=(c == 0), stop=(c == 6),
                                     skip_group_check=True)
            nc.gpsimd.tensor_mul(out=gv[:, :, D:2 * D], in0=hv,
                                 in1=gv[:, :, 0:D])
            nc.sync.dma_start(
                out=g_out[b, t00:t00 + nfull * P, D:3 * D].rearrange(
                    "(k p) x -> p k x", p=P),
                in_=gt[:, 0:nfull * 2 * D].rearrange("p (k x) -> p k x",
                                                     x=2 * D))
            if nfull < k:
                nc.sync.dma_start(
                    out=g_out[b, (c0 + nfull) * P:T, D:3 * D],
                    in_=gt[:lrows, nfull * 2 * D:k * 2 * D])

        def stage_hstores(b, S):
            # g cols 0:200 (= h) stream straight from htile — always-ready
            # DMA work that fills store-queue stalls
            htile = S['htile']
            nc.sync.dma_start(
                out=g_out[b, 0:6 * P, 0:D].rearrange("(k p) x -> p k x", p=P),
                in_=htile[:, 0:6 * DS].rearrange(
                    "p (k d) -> p k d", d=DS)[:, :, 0:D])
            nc.sync.dma_start(out=g_out[b, 6 * P:T, 0:D],
                              in_=htile[:T - 6 * P, 6 * DS:6 * DS + D])

        def stage_tail(b, S):
            htile, yT_ps = S['htile'], S['yT_ps']
            yT_sb = S['yT_sb']
            yT2_ps = S['yT2_ps']
            # ---- batch tail: q2c. Transpose the column accumulator back to a
            # row [1, 201] whose col 200 is sum(e); normalize and broadcast ----
            nc.vector.tensor_copy(out=yT_sb[:, 0:1], in_=yT_ps[:, :])
            nc.vector.tensor_copy(out=yT_sb[:, 1:2], in_=yT2_ps[:, :])
            ytp = ps_c2q.tile([P, DS + 1], f32, tag="c2q", name="ytp")
            nc.tensor.matmul(ytp[:1, 0:P], lhsT=yT_sb[:P, 0:1],
                             rhs=identity[:P, :P], is_transpose=True,
                             skip_group_check=True)
            nc.tensor.matmul(ytp[:1, P:DS], lhsT=yT_sb[:DS - P, 1:2],
                             rhs=identity[:DS - P, :DS - P], is_transpose=True,
                             skip_group_check=True)
            y_row = bsmall.tile([1, DS], f32, tag="q2c", name="y_row")
            nc.scalar.copy(out=y_row[:1, 0:DS], in_=ytp[:1, 0:DS])
            Sinv = bsmall.tile([1, 1], f32, tag="Sinv")
            nc.vector.reciprocal(out=Sinv[:1, :], in_=y_row[:1, D:DS])
            q2c_sb = bsmall.tile([1, D], f32, tag="q2cs")
            nc.vector.tensor_scalar_mul(out=q2c_sb[:1, :], in0=y_row[:1, 0:D],
                                        scalar1=Sinv[:1, :])
            q2cb_ps = ps_acc.tile([P, D], f32, tag="accA", bufs=2)
            nc.tensor.matmul(q2cb_ps[:, :], lhsT=ones_row[:1, :], rhs=q2c_sb[:1, :],
                             start=True, stop=True)
            q2cb_sb = bsmall.tile([P, D], f32, tag="q2cb")
            nc.scalar.copy(out=q2cb_sb[:, :], in_=q2cb_ps[:, :])

            hq_all = hqpool.tile([P, 7 * D], f32, tag="hq")
            q2cb_b3 = bass.AP(tensor=q2cb_sb.tensor, offset=q2cb_sb.offset,
                              ap=[q2cb_sb.ap[0], [0, 3], q2cb_sb.ap[1]])
            nc.vector.tensor_mul(
                out=hq_all[:, 0:3 * D].rearrange("p (n d) -> p n d", d=D),
                in0=htile[:, 0:3 * DS].rearrange("p (n d) -> p n d", d=DS)[:, :, 0:D],
                in1=q2cb_b3)
            nc.gpsimd.tensor_mul(
                out=hq_all[:, 3 * D:6 * D].rearrange("p (n d) -> p n d", d=D),
                in0=htile[:, 3 * DS:6 * DS].rearrange("p (n d) -> p n d", d=DS)[:, :, 0:D],
                in1=q2cb_b3)
            nc.vector.tensor_mul(out=hq_all[0:T - 6 * P, 6 * D:7 * D],
                                 in0=htile[0:T - 6 * P, 6 * DS:6 * DS + D],
                                 in1=q2cb_sb[0:T - 6 * P, :])
            nc.sync.dma_start(
                out=g_out[b, 0:6 * P, 3 * D:4 * D].rearrange("(n p) d -> p n d", p=P),
                in_=hq_all[:, 0:6 * D].rearrange("p (n d) -> p n d", d=D))
            nc.sync.dma_start(out=g_out[b, 6 * P:T, 3 * D:4 * D],
                              in_=hq_all[0:T - 6 * P, 6 * D:7 * D])

        PAIRS = ((0, 1), (2, 3), (4, 5, 6))
        HGROUPS = ((0, 1), (2, 3), (4, 5), (6,))

        NPRE = 5   # batches loaded before compute issue starts

        def run_all():
            # loads for the first NPRE batches go up front; later loads are
            # woven between store issues so the DMA stream never starves.
            # Weight loads interleave with the h loads so their HWDGE
            # generation hides under the big transfers.
            load_body(0)
            load_weights_a()
            warm_pe()
            load_body(1)
            load_weights_b()
            for b in range(2, NPRE):
                load_body(b)
            # software-pipelined issue order: weave batch b+1's load-only-
            # dependent prep between batch b's dependent pair stages so each
            # engine's in-order stream always has independent work to run
            # while a chain stalls
            Ss = [dict() for _ in range(BC)]
            stage_uprep(0, Ss[0])
            stage_hstores(0, Ss[0])
            stage_hT(0, Ss[0], HGROUPS[:2])
            stage_hT(0, Ss[0], HGROUPS[2:])
            for b in range(BC - 2):
                stage_pair(b, Ss[b], PAIRS[0])
                stage_pair(b, Ss[b], PAIRS[1])
                if b + NPRE < BC:
                    load_body(b + NPRE)
                stage_uprep(b + 1, Ss[b + 1])
                stage_hT(b + 1, Ss[b + 1], HGROUPS[:2])
                stage_pair(b, Ss[b], PAIRS[2])
                stage_tail(b, Ss[b])
                stage_hT(b + 1, Ss[b + 1], HGROUPS[2:])
            # epilogue: the last two batches interleave with each other at
            # pair granularity so the drain phase still has independent work
            a, z = BC - 2, BC - 1
            stage_pair(a, Ss[a], PAIRS[0])
            stage_uprep(z, Ss[z])
            stage_pair(a, Ss[a], PAIRS[1])
            stage_hT(z, Ss[z], HGROUPS[:2])
            stage_pair(a, Ss[a], PAIRS[2])
            stage_hT(z, Ss[z], HGROUPS[2:])
            stage_pair(z, Ss[z], PAIRS[0])
            stage_pair(z, Ss[z], PAIRS[1])
            stage_tail(a, Ss[a])
            stage_pair(z, Ss[z], PAIRS[2])
            stage_tail(z, Ss[z])

        if reps == 1:
            run_all()
        else:
            with tc.For_i(0, reps, 1):
                run_all()

    return nc


def kernel(h, u, w_h, b_h, w_u, b_u, w_hu, b_hu):
    from concourse.bass_utils import run_bass_kernel_spmd

    if "nc" not in _cache:
        nc = _build()
        _split_multi_waits(nc)
        _cache["nc"] = nc
    nc = _cache["nc"]

    h = np.ascontiguousarray(h, dtype=np.float32)
    u = np.ascontiguousarray(u, dtype=np.float32)
    w_h = np.ascontiguousarray(w_h, dtype=np.float32)
    w_u = np.ascontiguousarray(w_u, dtype=np.float32)
    w_hu = np.ascontiguousarray(w_hu, dtype=np.float32)

    core_ids = list(range(NCORES))
    in_maps = []
    for i in core_ids:
        in_maps.append({
            "h": h[i * BC:(i + 1) * BC],
            "u": u[i * BC:(i + 1) * BC],
            "w_h": w_h,
            "w_u": w_u,
            "w_hu": w_hu,
        })
    res = run_bass_kernel_spmd(nc, in_maps, core_ids)
    _cache["last_results"] = res
    return np.concatenate([res.results[i]["g"] for i in core_ids], axis=0)

